# revision 23
# baseline (speedup 1.0000x reference)
"""Trainium2 Bass kernel for DeepMinAttLSTM (4x minLSTM + MHSA + last-step FC).

Strategy:
  - Data-parallel over batch: 16 batches -> 8 cores x 2 batches.
  - Everything on device is kept feature-major: activations live as
    X^T [H=1024 (8 partition-tiles of 128), B*S=2048 free] in bf16.
  - Per layer: 3 gate matmuls (W^T stationary, X^T moving, fp32 PSUM),
    sigmoid gates on ACT, fp/add gate math on DVE (reciprocal_approx_fast),
    and the sequential minLSTM recurrence via the DVE tensor_tensor_scan
    instruction (state fp32) along the free (time) dimension.
  - The final output only uses out[:, -1, :], so attention collapses to the
    last query position AND the K/V full-sequence GEMMs are reassociated
    away:
      scores_s = (Wk_j^T q_j) . h4_s   (k-bias drops: softmax shift-invar.)
      z_j      = sum_s e_s * h4_s      (via PE-transposed h4)
      o_j      = Wv_j^T z_j / den + bv_j
    This removes ~2*[2048,1024]x[1024,1024] matmuls per core (~110us PE).
  - All matmuls in bf16 with fp32 accumulation (predicted rel err ~7e-3).
"""

import math

import numpy as np
import ml_dtypes

BF16 = ml_dtypes.bfloat16

P = 128
H = 1024
S = 1024
B = 16
NCORES = 8
BC = B // NCORES          # batches per core
BS = BC * S               # 2048 free columns per core
KO = H // P               # 8 feature partition-tiles
NH = 8
DH = H // NH              # 128
O = 256
L = 4
QSCALE = 1.0 / math.sqrt(DH)

_CACHE = {}


def _build_nc():
    import concourse.mybir as mybir
    import concourse.tile as tile
    from concourse import bacc
    from concourse import masks

    DT = mybir.dt.bfloat16
    F32 = mybir.dt.float32
    AFT = mybir.ActivationFunctionType
    OP = mybir.AluOpType

    nc = bacc.Bacc("TRN2", target_bir_lowering=False, debug=False,
                   num_devices=NCORES)

    xT = nc.dram_tensor("xT", [P, KO * BS], DT, kind="ExternalInput").ap()
    gw = nc.dram_tensor("gw", [3 * L * P, KO * H], DT, kind="ExternalInput").ap()
    gb = nc.dram_tensor("gb", [P, 3 * L * KO], F32, kind="ExternalInput").ap()
    # ip = [ipq (KO*H) | wkd (NH*H) | wvT (KO*H)]
    ip = nc.dram_tensor("ip", [P, KO * 3 * H], DT, kind="ExternalInput").ap()
    ipb = nc.dram_tensor("ipb", [P, BC * KO], F32, kind="ExternalInput").ap()
    vb = nc.dram_tensor("vb", [P, BC * NH], F32, kind="ExternalInput").ap()
    ow = nc.dram_tensor("ow", [P, KO * H], DT, kind="ExternalInput").ap()
    ob = nc.dram_tensor("ob", [P, BC * KO], F32, kind="ExternalInput").ap()
    fcw = nc.dram_tensor("fcw", [P, KO * O], DT, kind="ExternalInput").ap()
    fcb = nc.dram_tensor("fcb", [P, O // P], F32, kind="ExternalInput").ap()
    outT = nc.dram_tensor("outT", [O, BC], F32, kind="ExternalOutput").ap()

    IPQ = 0           # ip column offsets
    WKD = KO * H
    WVT = 2 * KO * H

    with tile.TileContext(nc) as tc:
        with (
            tc.tile_pool(name="constp", bufs=1) as constp,
            tc.tile_pool(name="hbuf", bufs=2) as hp,
        ):
            gb_sb = constp.tile([P, 3 * L * KO], F32)
            nc.sync.dma_start(gb_sb[:], gb[:])
            ones_col = constp.tile([P, 1], DT)
            nc.vector.memset(ones_col[:], 1.0)
            ones_row = constp.tile([1, P], F32)
            nc.vector.memset(ones_row[:], 1.0)
            ident = constp.tile([P, P], DT)
            masks.make_identity(nc, ident[:])

            # layer-0 input is CHUNK-major ([ch, ko, 512]) so each chunk
            # DMA is one contiguous 8KB-per-partition run (DMA packet
            # efficiency); layers 1+ use the ko-major h layout.
            X = hp.tile([P, KO * BS], DT, tag="hbuf", name="xT_sb")

            # gw pool also carries the in_proj tiles (allocated during
            # layer 3 so their DMAs overlap layer-3 compute); it stays open
            # through the attention section.
            gwp_mgr = tc.tile_pool(name="gwp", bufs=5)
            gwp = gwp_mgr.__enter__()
            owp_mgr = tc.tile_pool(name="owp", bufs=1)
            owp = owp_mgr.__enter__()
            ip_tiles = {}
            ow_sb = owp.tile([P, KO * H], DT)
            fcw_sb = owp.tile([P, KO * O], DT)
            ipb_sb = constp.tile([P, BC * KO], F32)
            vb_sb = constp.tile([P, BC * NH], F32)
            ob_sb = constp.tile([P, BC * KO], F32)
            fcb_sb = constp.tile([P, O // P], F32)

            def chunked_dma(eng, dst, src, nch=4):
                cols = dst.shape[1]
                per = cols // nch
                for c in range(nch):
                    eng.dma_start(dst[:, c * per:(c + 1) * per],
                                  src[:, c * per:(c + 1) * per])

            # ---------------- minLSTM layers ----------------
            with (
                tc.tile_pool(name="fpp", bufs=2) as fpp,
                tc.tile_pool(name="addp", bufs=2) as addp,
                tc.tile_pool(name="tmpp", bufs=2) as tmpp,
                tc.tile_pool(name="psA", bufs=6, space="PSUM") as psA,
            ):
                for l in range(L):
                    gws = []
                    for g in range(3):
                        gws.append(gwp.tile([P, KO * H], DT, tag="gw",
                                            name=f"gw_{l}_{g}"))
                    if l == 0:
                        # Startup is total-HBM-bandwidth bound. Layer-0
                        # weights are stored NO-major (host prep), so the
                        # first no-group only needs its own 0.75MB of
                        # weights: issue x0, then per-no weight slices
                        # interleaved with the later x chunks, striped
                        # round-robin over the three DMA rings. Compute
                        # starts ~1.75MB in instead of ~7MB.
                        CW = KO * 512            # one x chunk = 4096 cols
                        need = [(X[:, 0:CW // 2], xT[:, 0:CW // 2]),
                                (X[:, CW // 2:CW], xT[:, CW // 2:CW])]

                        def wno(no):
                            for g in range(3):
                                need.append(
                                    (gws[g][:, no * H:(no + 1) * H],
                                     gw[g * P:(g + 1) * P,
                                        no * H:(no + 1) * H]))
                        wno(0)
                        need.append((X[:, CW:2 * CW], xT[:, CW:2 * CW]))
                        wno(1)
                        need.append((X[:, 2 * CW:3 * CW],
                                     xT[:, 2 * CW:3 * CW]))
                        wno(2)
                        need.append((X[:, 3 * CW:4 * CW],
                                     xT[:, 3 * CW:4 * CW]))
                        for no in range(3, KO):
                            wno(no)
                        rings = [nc.sync, nc.scalar, nc.gpsimd]
                        for i, (dst, srcap) in enumerate(need):
                            rings[i % 3].dma_start(dst, srcap)
                        # attention weights/biases: plenty of DMA slack in
                        # the layer-0 window; keeps the l2->l3 boundary and
                        # the layer-3 window free for gw/in_proj traffic
                        chunked_dma(nc.sync, ow_sb[:], ow[:])
                        nc.scalar.dma_start(fcw_sb[:], fcw[:])
                        nc.scalar.dma_start(ipb_sb[:], ipb[:])
                        nc.scalar.dma_start(vb_sb[:], vb[:])
                        nc.scalar.dma_start(ob_sb[:], ob[:])
                        nc.scalar.dma_start(fcb_sb[:], fcb[:])
                    else:
                        for g in range(3):
                            lg = l * 3 + g
                            chunked_dma(nc.gpsimd, gws[g][:],
                                        gw[lg * P:(lg + 1) * P, :])
                    if l == L - 1:
                        # in_proj tiles ride the gw pool rotation: ipq/wkd
                        # reuse buffers freed at the start of layer 3; wvT
                        # reuses l3g0's buffer (freed at layer 3's end, and
                        # wvT is only needed late in the attention tail).
                        for nm, off in (("ipq", IPQ), ("wkd", WKD),
                                        ("wvT", WVT)):
                            t = gwp.tile([P, KO * H], DT, tag="gw", name=nm)
                            chunked_dma(nc.gpsimd, t[:],
                                        ip[:, off: off + KO * H])
                            ip_tiles[nm] = t
                    h_out = hp.tile([P, KO * BS], DT, tag="hbuf", name=f"h_{l}")
                    for no in range(KO):
                        fp_t = fpp.tile([P, BS], DT, tag="fp",
                                        name=f"fp_{l}_{no}")
                        add_t = addp.tile([P, BS], DT, tag="add",
                                          name=f"add_{l}_{no}")
                        for ch in range(4):
                            m0 = ch * 512
                            psF = psA.tile([P, 512], F32, tag="ps", name="psF")
                            psI = psA.tile([P, 512], F32, tag="ps", name="psI")
                            psH = psA.tile([P, 512], F32, tag="ps", name="psH")
                            for g, ps in ((0, psF), (1, psI), (2, psH)):
                                for ko in range(KO):
                                    if l == 0:
                                        xs = ch * KO * 512 + ko * 512
                                        wsl = no * H + ko * P
                                    else:
                                        xs = ko * BS + m0
                                        wsl = ko * H + no * P
                                    nc.tensor.matmul(
                                        ps[:],
                                        gws[g][:, wsl: wsl + P],
                                        X[:, xs: xs + 512],
                                        start=(ko == 0), stop=(ko == KO - 1))
                            f_t = tmpp.tile([P, 512], DT, tag="f_t", name="f_t")
                            i_t = tmpp.tile([P, 512], DT, tag="i_t", name="i_t")
                            d_t = tmpp.tile([P, 512], F32, tag="d_t", name="d_t")
                            r_t = tmpp.tile([P, 512], F32, tag="r_t", name="r_t")
                            t1 = tmpp.tile([P, 512], F32, tag="t1", name="t1",
                                           bufs=1)
                            bF = gb_sb[:, (l * 3 + 0) * KO + no:
                                       (l * 3 + 0) * KO + no + 1]
                            bI = gb_sb[:, (l * 3 + 1) * KO + no:
                                       (l * 3 + 1) * KO + no + 1]
                            bH = gb_sb[:, (l * 3 + 2) * KO + no:
                                       (l * 3 + 2) * KO + no + 1]
                            nc.scalar.activation(f_t[:], psF[:], AFT.Sigmoid,
                                                 bias=bF)
                            nc.scalar.activation(i_t[:], psI[:], AFT.Sigmoid,
                                                 bias=bI)
                            nc.vector.tensor_add(d_t[:], f_t[:], i_t[:])
                            nc.vector.reciprocal_approx_fast(r_t[:], d_t[:])
                            nc.vector.tensor_mul(
                                fp_t[:, m0:m0 + 512], f_t[:], r_t[:])
                            # t1 = (ht_psum + bh) * r
                            nc.vector.scalar_tensor_tensor(
                                t1[:], psH[:], bH, r_t[:],
                                op0=OP.add, op1=OP.mult)
                            nc.vector.tensor_mul(
                                add_t[:, m0:m0 + 512], t1[:], i_t[:])
                            # recurrence for batch b runs as one [128,1024]
                            # scan once both of its chunks' gate math exists
                            # (scan instr overhead ~550ns, so fewer+bigger)
                            b, half = ch // 2, ch % 2
                            base = no * BS + b * S
                            if half == 1:
                                nc.vector.tensor_tensor_scan(
                                    h_out[:, base: base + S],
                                    fp_t[:, b * S: (b + 1) * S],
                                    add_t[:, b * S: (b + 1) * S],
                                    initial=0.0, op0=OP.mult, op1=OP.add)
                    X = h_out

            h4 = X

            # ---------------- attention (last query position only) ----------
            with (
                tc.tile_pool(name="smallp", bufs=1) as smallp,
            ):
                h4T = hp.tile([P, BC * KO * H], DT, tag="hbuf",
                              name="h4T")  # [s-part, (b, st, h)]
                lastq = smallp.tile([P, BC * KO], DT)
                q_sb = smallp.tile([P, BC * KO], DT)
                qe_sb = smallp.tile([P, KO * 2 * NH], DT)  # col no*16+b*8+j
                eT_sb = smallp.tile([P, BC * KO * NH], DT)  # col (b*8+kt)*8+j
                z_sb = smallp.tile([P, KO * 2 * NH], DT)   # col ko*16+j*2+b
                e_n = smallp.tile([NH, BC * S], DT)        # [j, (b,s)]
                acc2 = smallp.tile([NH, 2 * BC], F32)      # col b*2+ch
                dL = smallp.tile([NH, BC], F32)
                drL = smallp.tile([NH, BC], F32)
                O_last = smallp.tile([P, 2 * KO], DT)      # col j*BC+b
                out_last = smallp.tile([P, 2 * KO], DT)
                res_sb = smallp.tile([P, 2 * (O // P)], F32)

                # h4 columns at the last timestep (per ko-tile, per batch)
                for ko in range(KO):
                    for b in range(BC):
                        nc.vector.tensor_copy(
                            lastq[:, ko * BC + b: ko * BC + b + 1],
                            h4[:, ko * BS + b * S + S - 1:
                               ko * BS + b * S + S])

                with (
                    tc.tile_pool(name="psK", bufs=2, space="PSUM") as psK,
                    tc.tile_pool(name="psB", bufs=1, space="PSUM") as psB,
                ):
                    # ---- h4T: transpose h4 into [s-part, (b, st, h)] ----
                    # (psum small-tile budget: mm(2) + den(1) + bc(1) + o(1)
                    #  = 5 banks alongside psK's 3)
                    for b in range(BC):
                        for st in range(KO):
                            pst = psK.tile([P, H], DT, tag="pst", name="pst")
                            for hb in range(KO):
                                nc.tensor.transpose(
                                    pst[:, hb * P:(hb + 1) * P],
                                    h4[:, hb * BS + b * S + st * P:
                                       hb * BS + b * S + (st + 1) * P],
                                    ident[:])
                            dst = h4T[:, (b * KO + st) * H:
                                      (b * KO + st + 1) * H]
                            # alternate evacuation engine to halve the
                            # psum->sbuf stage latency
                            if st % 2 == 0:
                                nc.scalar.activation(dst, pst[:], AFT.Copy)
                            else:
                                nc.vector.tensor_copy(dst, pst[:])

                    # ---- q at the last position (QSCALE lives in wkd) ----
                    ps_q = psB.tile([P, 2 * NH], F32, tag="mm",
                                    name="ps_q", bufs=2)
                    for nt in range(KO):
                        for ko in range(KO):
                            nc.tensor.matmul(
                                ps_q[:, nt * BC: (nt + 1) * BC],
                                ip_tiles["ipq"][:, ko * H + nt * P:
                                                ko * H + (nt + 1) * P],
                                lastq[:, ko * BC: (ko + 1) * BC],
                                start=(ko == 0), stop=(ko == KO - 1))
                    nc.vector.tensor_add(q_sb[:], ps_q[:], ipb_sb[:])

                    # ---- qe_j = Wk_j^T q_j  (per head, [H] each) ----
                    for no in range(KO):
                        ps_qe = psB.tile([P, 2 * NH], F32, tag="mm",
                                         name="ps_qe", bufs=2)
                        for j in range(NH):
                            nc.tensor.matmul(
                                ps_qe[:, j * BC: (j + 1) * BC],
                                ip_tiles["wkd"][:, j * H + no * P:
                                                j * H + (no + 1) * P],
                                q_sb[:, j * BC: (j + 1) * BC],
                                start=True, stop=True)
                        nc.scalar.activation(
                            qe_sb[:, no * 2 * NH: (no + 1) * 2 * NH],
                            ps_qe[:], AFT.Copy)

                    # ---- scores in [j-part, s-free]: qe stationary (cheap
                    # ldweights), h4 moving 512-wide; softmax denominator is
                    # then a per-partition scalar (j on partitions) ----
                    for b in range(BC):
                        for ch2 in range(2):
                            ps_sc = psB.tile([P, 512], F32, tag="sc",
                                             name="ps_sc", bufs=2)
                            for ko in range(KO):
                                nc.tensor.matmul(
                                    ps_sc[:NH, :],
                                    qe_sb[:, ko * 2 * NH + b:
                                          (ko + 1) * 2 * NH: BC],
                                    h4[:, ko * BS + b * S + ch2 * 512:
                                       ko * BS + b * S + (ch2 + 1) * 512],
                                    start=(ko == 0), stop=(ko == KO - 1))
                            bc2 = b * 2 + ch2
                            nc.scalar.activation(
                                e_n[:, bc2 * 512: (bc2 + 1) * 512],
                                ps_sc[:NH, :], AFT.Exp,
                                accum_out=acc2[:, bc2: bc2 + 1])
                    # denominators + normalize (j is the partition dim here)
                    nc.vector.tensor_add(dL[:], acc2[:, 0::2], acc2[:, 1::2])
                    nc.vector.reciprocal(drL[:], dL[:])
                    for b in range(BC):
                        nc.vector.tensor_scalar_mul(
                            e_n[:, b * S: (b + 1) * S],
                            e_n[:, b * S: (b + 1) * S], drL[:, b: b + 1])
                    # transpose e back to [s-part, j] tiles for the z stage
                    for b in range(BC):
                        ps_et = psB.tile([P, KO * NH], DT, tag="et",
                                         name="ps_et", bufs=1)
                        for st in range(KO):
                            nc.tensor.transpose(
                                ps_et[:, st * NH: (st + 1) * NH],
                                e_n[:NH, b * S + st * P: b * S + (st + 1) * P],
                                ident[:NH, :NH])
                        nc.scalar.activation(
                            eT_sb[:, b * KO * NH: (b + 1) * KO * NH],
                            ps_et[:], AFT.Copy)

                    # ---- z = sum_s e_s * h4_s (unnormalized) ----
                    for b in range(BC):
                        for ko in range(KO):
                            ps_zt = psB.tile([P, 2 * NH], F32, tag="mm",
                                             name="ps_z", bufs=2)
                            ps_z = ps_zt[:, :NH]
                            for st in range(KO):
                                nc.tensor.matmul(
                                    ps_z[:],
                                    h4T[:, (b * KO + st) * H + ko * P:
                                        (b * KO + st) * H + (ko + 1) * P],
                                    eT_sb[:, (b * KO + st) * NH:
                                          (b * KO + st + 1) * NH],
                                    start=(st == 0), stop=(st == KO - 1))
                            # strided write: z col = ko*16 + j*2 + b
                            nc.scalar.activation(
                                z_sb[:, ko * 2 * NH + b:
                                     (ko + 1) * 2 * NH: BC],
                                ps_z[:], AFT.Copy)

                    # ---- o_j = Wv_j^T z_j, normalized + v bias ----
                    ps_o = psB.tile([P, 2 * NH], F32, tag="o", name="ps_o")
                    for j in range(NH):
                        for ko in range(KO):
                            nc.tensor.matmul(
                                ps_o[:, j * BC: (j + 1) * BC],
                                ip_tiles["wvT"][:, ko * H + j * P:
                                                ko * H + (j + 1) * P],
                                z_sb[:, ko * 2 * NH + j * BC:
                                     ko * 2 * NH + (j + 1) * BC],
                                start=(ko == 0), stop=(ko == KO - 1))
                    nc.vector.tensor_add(O_last[:], ps_o[:], vb_sb[:])

                    # ---- out projection at last position + residual ----
                    ps_p = psB.tile([P, 2 * NH], F32, tag="mm",
                                    name="ps_p", bufs=2)
                    for no in range(KO):
                        for ko in range(KO):
                            nc.tensor.matmul(
                                ps_p[:, no * BC: (no + 1) * BC],
                                ow_sb[:, ko * H + no * P: ko * H + (no + 1) * P],
                                O_last[:, ko * BC: (ko + 1) * BC],
                                start=(ko == 0), stop=(ko == KO - 1))
                    nc.vector.tensor_add(out_last[:], ps_p[:], ob_sb[:])
                    nc.vector.tensor_add(out_last[:], out_last[:], lastq[:])
                    # ---- final fc ----
                    for ot in range(O // P):
                        ps_ft = psB.tile([P, 2 * NH], F32, tag="mm",
                                         name="ps_f", bufs=2)
                        ps_f = ps_ft[:, :BC]
                        for ko in range(KO):
                            nc.tensor.matmul(
                                ps_f[:],
                                fcw_sb[:, ko * O + ot * P: ko * O + (ot + 1) * P],
                                out_last[:, ko * BC: (ko + 1) * BC],
                                start=(ko == 0), stop=(ko == KO - 1))
                        nc.scalar.activation(
                            res_sb[:, ot * BC: (ot + 1) * BC], ps_f[:],
                            AFT.Identity, bias=fcb_sb[:, ot:ot + 1])
                    outT_v = outT.rearrange("(o p) b -> p o b", o=O // P)
                    res_v = res_sb.rearrange("p (o b) -> p o b", o=O // P)
                    nc.sync.dma_start(outT_v[:, :, :], res_v[:, :, :])

            owp_mgr.__exit__(None, None, None)
            gwp_mgr.__exit__(None, None, None)

    nc.compile()
    return nc


def _feature_major(w_t):
    """[H_in, N] (already transposed weight) -> device layout [128, KO*N]."""
    hin, n = w_t.shape
    ko = hin // P
    return np.ascontiguousarray(
        w_t.reshape(ko, P, n).transpose(1, 0, 2).reshape(P, ko * n))


def _prep_inputs(x, Wf, bf, Wi, bi, Wh, bh, in_proj_w, in_proj_b, out_w,
                 out_b, fc_w, fc_b):
    gws = []
    gbs = []
    for l in range(L):
        for W, bias in ((Wf[l], bf[l]), (Wi[l], bi[l]), (Wh[l], bh[l])):
            gws.append(_feature_major(W.T.astype(np.float32)).astype(BF16))
            gbs.append(bias.reshape(KO, P).T.astype(np.float32))
    for g in range(3):       # layer-0 gates: NO-major for startup DMA
        gws[g] = np.ascontiguousarray(
            gws[g].reshape(P, KO, KO, P).transpose(0, 2, 1, 3)
            .reshape(P, KO * H))
    gw = np.concatenate(gws, axis=0)                     # [12*128, KO*H]
    gb = np.concatenate(gbs, axis=1)                     # [128, 12*KO]
    # ip = [ipq | wkd | wvT]
    ipq = _feature_major(in_proj_w[:H].T.astype(np.float32))
    wkd = np.ascontiguousarray(
        in_proj_w[H:2 * H].astype(np.float32)
        .reshape(NH, P, H).transpose(1, 0, 2).reshape(P, NH * H)) * QSCALE
    wvT = _feature_major(in_proj_w[2 * H:3 * H].T.astype(np.float32))
    ip = np.concatenate([ipq, wkd, wvT], axis=1).astype(BF16)
    ipb = np.repeat(in_proj_b[:H].reshape(KO, P).T.astype(np.float32),
                    BC, axis=1)                          # col nt*BC+b
    vbv = np.repeat(in_proj_b[2 * H:].reshape(NH, P).T.astype(np.float32),
                    BC, axis=1)                          # col j*BC+b
    owp = _feature_major(out_w.T.astype(np.float32)).astype(BF16)
    obv = np.repeat(out_b.reshape(KO, P).T.astype(np.float32), BC, axis=1)
    fcwp = _feature_major(fc_w.T.astype(np.float32)).astype(BF16)
    fcbv = fc_b.reshape(O // P, P).T.astype(np.float32)
    shared = dict(gw=gw, gb=np.ascontiguousarray(gb),
                  ip=ip, ipb=np.ascontiguousarray(ipb),
                  vb=np.ascontiguousarray(vbv), ow=owp,
                  ob=np.ascontiguousarray(obv), fcw=fcwp,
                  fcb=np.ascontiguousarray(fcbv))
    in_maps = []
    for c in range(NCORES):
        shard = x[c * BC:(c + 1) * BC]                   # [BC, S, H]
        xt = shard.transpose(2, 0, 1).reshape(H, BS)     # [H, BS]
        xt = _feature_major(xt)                          # [128, KO*BS]
        # chunk-major: [p, ch(4), ko(8), 512] so chunk DMAs are contiguous
        xt = np.ascontiguousarray(
            xt.reshape(P, KO, 4, 512).transpose(0, 2, 1, 3)
            .reshape(P, KO * BS)).astype(BF16)
        in_maps.append(dict(shared, xT=xt))
    return in_maps


def kernel(x, Wf, bf, Wi, bi, Wh, bh, in_proj_w, in_proj_b, out_w, out_b,
           fc_w, fc_b):
    from concourse.bass_utils import run_bass_kernel_spmd

    x, Wf, bf, Wi, bi, Wh, bh = (np.asarray(t) for t in
                                 (x, Wf, bf, Wi, bi, Wh, bh))
    in_proj_w, in_proj_b, out_w, out_b, fc_w, fc_b = (
        np.asarray(t) for t in (in_proj_w, in_proj_b, out_w, out_b,
                                fc_w, fc_b))
    if "nc" not in _CACHE:
        _CACHE["nc"] = _build_nc()
    nc = _CACHE["nc"]
    in_maps = _prep_inputs(x, Wf, bf, Wi, bi, Wh, bh, in_proj_w, in_proj_b,
                           out_w, out_b, fc_w, fc_b)
    res = run_bass_kernel_spmd(nc, in_maps, core_ids=list(range(NCORES)))
    _CACHE["last_results"] = res
    out = np.empty((B, O), np.float32)
    for c in range(NCORES):
        outT = res.results[c]["outT"]                    # [O, BC]
        for b in range(BC):
            out[c * BC + b] = outT[:, b]
    return out


# revision 27
# speedup vs baseline: 1.0021x; 1.0021x over previous
"""Trainium2 Bass kernel for DeepMinAttLSTM (4x minLSTM + MHSA + last-step FC).

Strategy:
  - Data-parallel over batch: 16 batches -> 8 cores x 2 batches.
  - Everything on device is kept feature-major: activations live as
    X^T [H=1024 (8 partition-tiles of 128), B*S=2048 free] in bf16.
  - Per layer: 3 gate matmuls (W^T stationary, X^T moving, fp32 PSUM),
    sigmoid gates on ACT, fp/add gate math on DVE (reciprocal_approx_fast),
    and the sequential minLSTM recurrence via the DVE tensor_tensor_scan
    instruction (state fp32) along the free (time) dimension.
  - The final output only uses out[:, -1, :], so attention collapses to the
    last query position AND the K/V full-sequence GEMMs are reassociated
    away:
      scores_s = (Wk_j^T q_j) . h4_s   (k-bias drops: softmax shift-invar.)
      z_j      = sum_s e_s * h4_s      (via PE-transposed h4)
      o_j      = Wv_j^T z_j / den + bv_j
    This removes ~2*[2048,1024]x[1024,1024] matmuls per core (~110us PE).
  - All matmuls in bf16 with fp32 accumulation (predicted rel err ~7e-3).
"""

import math

import numpy as np
import ml_dtypes

BF16 = ml_dtypes.bfloat16

P = 128
H = 1024
S = 1024
B = 16
NCORES = 8
BC = B // NCORES          # batches per core
BS = BC * S               # 2048 free columns per core
KO = H // P               # 8 feature partition-tiles
NH = 8
DH = H // NH              # 128
O = 256
L = 4
QSCALE = 1.0 / math.sqrt(DH)

_CACHE = {}


def _build_nc():
    import concourse.mybir as mybir
    import concourse.tile as tile
    from concourse import bacc
    from concourse import masks

    DT = mybir.dt.bfloat16
    F32 = mybir.dt.float32
    AFT = mybir.ActivationFunctionType
    OP = mybir.AluOpType

    nc = bacc.Bacc("TRN2", target_bir_lowering=False, debug=False,
                   num_devices=NCORES)

    xT = nc.dram_tensor("xT", [P, KO * BS], DT, kind="ExternalInput").ap()
    gw = nc.dram_tensor("gw", [3 * L * P, KO * H], DT, kind="ExternalInput").ap()
    gb = nc.dram_tensor("gb", [P, 3 * L * KO], F32, kind="ExternalInput").ap()
    # ip = [ipq (KO*H) | wkd (NH*H) | wvT (KO*H)]
    ip = nc.dram_tensor("ip", [P, KO * 3 * H], DT, kind="ExternalInput").ap()
    ipb = nc.dram_tensor("ipb", [P, BC * KO], F32, kind="ExternalInput").ap()
    vb = nc.dram_tensor("vb", [P, BC * NH], F32, kind="ExternalInput").ap()
    ow = nc.dram_tensor("ow", [P, KO * H], DT, kind="ExternalInput").ap()
    ob = nc.dram_tensor("ob", [P, BC * KO], F32, kind="ExternalInput").ap()
    fcw = nc.dram_tensor("fcw", [P, KO * O], DT, kind="ExternalInput").ap()
    fcb = nc.dram_tensor("fcb", [P, O // P], F32, kind="ExternalInput").ap()
    outT = nc.dram_tensor("outT", [O, BC], F32, kind="ExternalOutput").ap()

    IPQ = 0           # ip column offsets
    WKD = KO * H
    WVT = 2 * KO * H

    with tile.TileContext(nc) as tc:
        with (
            tc.tile_pool(name="constp", bufs=1) as constp,
            tc.tile_pool(name="hbuf", bufs=2) as hp,
        ):
            gb_sb = constp.tile([P, 3 * L * KO], F32)
            nc.sync.dma_start(gb_sb[:], gb[:])
            ones_col = constp.tile([P, 1], DT)
            nc.vector.memset(ones_col[:], 1.0)
            ones_row = constp.tile([1, P], F32)
            nc.vector.memset(ones_row[:], 1.0)
            ident = constp.tile([P, P], DT)
            masks.make_identity(nc, ident[:])

            # layer-0 input is CHUNK-major ([ch, ko, 512]) so each chunk
            # DMA is one contiguous 8KB-per-partition run (DMA packet
            # efficiency); layers 1+ use the ko-major h layout.
            X = hp.tile([P, KO * BS], DT, tag="hbuf", name="xT_sb")

            # gw pool also carries the in_proj tiles (allocated during
            # layer 3 so their DMAs overlap layer-3 compute); it stays open
            # through the attention section.
            gwp_mgr = tc.tile_pool(name="gwp", bufs=5)
            gwp = gwp_mgr.__enter__()
            owp_mgr = tc.tile_pool(name="owp", bufs=1)
            owp = owp_mgr.__enter__()
            ip_tiles = {}
            ow_sb = owp.tile([P, KO * H], DT)
            fcw_sb = owp.tile([P, KO * O], DT)
            ipb_sb = constp.tile([P, BC * KO], F32)
            vb_sb = constp.tile([P, BC * NH], F32)
            ob_sb = constp.tile([P, BC * KO], F32)
            fcb_sb = constp.tile([P, O // P], F32)

            def chunked_dma(eng, dst, src, nch=4):
                cols = dst.shape[1]
                per = cols // nch
                for c in range(nch):
                    eng.dma_start(dst[:, c * per:(c + 1) * per],
                                  src[:, c * per:(c + 1) * per])

            # ---------------- minLSTM layers ----------------
            with (
                tc.tile_pool(name="fpp", bufs=2) as fpp,
                tc.tile_pool(name="addp", bufs=2) as addp,
                tc.tile_pool(name="tmpp", bufs=2) as tmpp,
                tc.tile_pool(name="psA", bufs=6, space="PSUM") as psA,
            ):
                for l in range(L):
                    gws = []
                    for g in range(3):
                        gws.append(gwp.tile([P, KO * H], DT, tag="gw",
                                            name=f"gw_{l}_{g}"))
                    if l == 0:
                        # Startup is total-HBM-bandwidth bound: issue the
                        # transfers in NEED order (x0, g0, g1, g2, x1..x3),
                        # striped round-robin over the three DMA rings, so
                        # the first psum groups can start ~3MB in instead of
                        # waiting for everything.
                        CW = KO * 512            # one chunk = 4096 cols
                        need = [(X[:, 0:CW // 2], xT[:, 0:CW // 2]),
                                (X[:, CW // 2:CW], xT[:, CW // 2:CW])]
                        for g in range(3):
                            gsrc = gw[g * P:(g + 1) * P, :]
                            for c in range(4):
                                need.append((gws[g][:, c * 2048:
                                                    (c + 1) * 2048],
                                             gsrc[:, c * 2048:
                                                  (c + 1) * 2048]))
                        for ch in range(1, 4):
                            need.append((X[:, ch * CW:(ch + 1) * CW],
                                         xT[:, ch * CW:(ch + 1) * CW]))
                        rings = [nc.sync, nc.scalar, nc.gpsimd]
                        for i, (dst, srcap) in enumerate(need):
                            rings[i % 3].dma_start(dst, srcap)
                        # attention weights/biases: plenty of DMA slack in
                        # the layer-0 window; keeps the l2->l3 boundary and
                        # the layer-3 window free for gw/in_proj traffic
                        chunked_dma(nc.sync, ow_sb[:], ow[:])
                        nc.scalar.dma_start(fcw_sb[:], fcw[:])
                        nc.scalar.dma_start(ipb_sb[:], ipb[:])
                        nc.scalar.dma_start(vb_sb[:], vb[:])
                        nc.scalar.dma_start(ob_sb[:], ob[:])
                        nc.scalar.dma_start(fcb_sb[:], fcb[:])
                    else:
                        for g in range(3):
                            lg = l * 3 + g
                            chunked_dma(nc.gpsimd, gws[g][:],
                                        gw[lg * P:(lg + 1) * P, :])
                    if l == L - 1:
                        # in_proj tiles ride the gw pool rotation: ipq/wkd
                        # reuse buffers freed at the start of layer 3; wvT
                        # reuses l3g0's buffer (freed at layer 3's end, and
                        # wvT is only needed late in the attention tail).
                        for nm, off in (("ipq", IPQ), ("wkd", WKD),
                                        ("wvT", WVT)):
                            t = gwp.tile([P, KO * H], DT, tag="gw", name=nm)
                            chunked_dma(nc.gpsimd, t[:],
                                        ip[:, off: off + KO * H])
                            ip_tiles[nm] = t
                    h_out = hp.tile([P, KO * BS], DT, tag="hbuf", name=f"h_{l}")
                    for no in range(KO):
                        fp_t = fpp.tile([P, BS], DT, tag="fp",
                                        name=f"fp_{l}_{no}")
                        add_t = addp.tile([P, BS], DT, tag="add",
                                          name=f"add_{l}_{no}")
                        for ch in range(4):
                            m0 = ch * 512
                            psF = psA.tile([P, 512], F32, tag="ps", name="psF")
                            psI = psA.tile([P, 512], F32, tag="ps", name="psI")
                            psH = psA.tile([P, 512], F32, tag="ps", name="psH")
                            for g, ps in ((0, psF), (1, psI), (2, psH)):
                                for ko in range(KO):
                                    if l == 0:
                                        xs = ch * KO * 512 + ko * 512
                                    else:
                                        xs = ko * BS + m0
                                    nc.tensor.matmul(
                                        ps[:],
                                        gws[g][:, ko * H + no * P:
                                               ko * H + (no + 1) * P],
                                        X[:, xs: xs + 512],
                                        start=(ko == 0), stop=(ko == KO - 1))
                            f_t = tmpp.tile([P, 512], DT, tag="f_t", name="f_t")
                            i_t = tmpp.tile([P, 512], DT, tag="i_t", name="i_t")
                            d_t = tmpp.tile([P, 512], F32, tag="d_t", name="d_t")
                            r_t = tmpp.tile([P, 512], F32, tag="r_t", name="r_t")
                            t1 = tmpp.tile([P, 512], F32, tag="t1", name="t1",
                                           bufs=1)
                            bF = gb_sb[:, (l * 3 + 0) * KO + no:
                                       (l * 3 + 0) * KO + no + 1]
                            bI = gb_sb[:, (l * 3 + 1) * KO + no:
                                       (l * 3 + 1) * KO + no + 1]
                            bH = gb_sb[:, (l * 3 + 2) * KO + no:
                                       (l * 3 + 2) * KO + no + 1]
                            nc.scalar.activation(f_t[:], psF[:], AFT.Sigmoid,
                                                 bias=bF)
                            nc.scalar.activation(i_t[:], psI[:], AFT.Sigmoid,
                                                 bias=bI)
                            nc.vector.tensor_add(d_t[:], f_t[:], i_t[:])
                            nc.vector.reciprocal_approx_fast(r_t[:], d_t[:])
                            nc.vector.tensor_mul(
                                fp_t[:, m0:m0 + 512], f_t[:], r_t[:])
                            # t1 = (ht_psum + bh) * r
                            nc.vector.scalar_tensor_tensor(
                                t1[:], psH[:], bH, r_t[:],
                                op0=OP.add, op1=OP.mult)
                            nc.vector.tensor_mul(
                                add_t[:, m0:m0 + 512], t1[:], i_t[:])
                            # recurrence for batch b runs as one [128,1024]
                            # scan once both of its chunks' gate math exists
                            # (scan instr overhead ~550ns, so fewer+bigger).
                            # Exception: the last no-tiles of the last layer
                            # run chunk-chained scans so the attention
                            # transposes (which need every no-tile) unblock
                            # as early as possible.
                            b, half = ch // 2, ch % 2
                            base = no * BS + b * S
                            if l == L - 1 and no >= KO - 2:
                                if half == 0:
                                    nc.vector.tensor_tensor_scan(
                                        h_out[:, base: base + 512],
                                        fp_t[:, b * S: b * S + 512],
                                        add_t[:, b * S: b * S + 512],
                                        initial=0.0, op0=OP.mult, op1=OP.add)
                                else:
                                    nc.vector.tensor_tensor_scan(
                                        h_out[:, base + 512: base + S],
                                        fp_t[:, b * S + 512: (b + 1) * S],
                                        add_t[:, b * S + 512: (b + 1) * S],
                                        initial=h_out[:, base + 511:
                                                      base + 512],
                                        op0=OP.mult, op1=OP.add)
                            elif half == 1:
                                nc.vector.tensor_tensor_scan(
                                    h_out[:, base: base + S],
                                    fp_t[:, b * S: (b + 1) * S],
                                    add_t[:, b * S: (b + 1) * S],
                                    initial=0.0, op0=OP.mult, op1=OP.add)
                    X = h_out

            h4 = X

            # ---------------- attention (last query position only) ----------
            with (
                tc.tile_pool(name="smallp", bufs=1) as smallp,
            ):
                h4T = hp.tile([P, BC * KO * H], DT, tag="hbuf",
                              name="h4T")  # [s-part, (b, st, h)]
                lastq = smallp.tile([P, BC * KO], DT)
                q_sb = smallp.tile([P, BC * KO], DT)
                qe_sb = smallp.tile([P, KO * 2 * NH], DT)  # col no*16+b*8+j
                eT_sb = smallp.tile([P, BC * KO * NH], DT)  # col (b*8+kt)*8+j
                z_sb = smallp.tile([P, KO * 2 * NH], DT)   # col ko*16+j*2+b
                e_n = smallp.tile([NH, BC * S], DT)        # [j, (b,s)]
                acc2 = smallp.tile([NH, 2 * BC], F32)      # col b*2+ch
                dL = smallp.tile([NH, BC], F32)
                drL = smallp.tile([NH, BC], F32)
                O_last = smallp.tile([P, 2 * KO], DT)      # col j*BC+b
                out_last = smallp.tile([P, 2 * KO], DT)
                res_sb = smallp.tile([P, 2 * (O // P)], F32)

                # h4 columns at the last timestep (per ko-tile, per batch)
                for ko in range(KO):
                    for b in range(BC):
                        nc.vector.tensor_copy(
                            lastq[:, ko * BC + b: ko * BC + b + 1],
                            h4[:, ko * BS + b * S + S - 1:
                               ko * BS + b * S + S])

                with (
                    tc.tile_pool(name="psK", bufs=2, space="PSUM") as psK,
                    tc.tile_pool(name="psB", bufs=1, space="PSUM") as psB,
                ):
                    # ---- h4T: transpose h4 into [s-part, (b, st, h)] ----
                    # (psum small-tile budget: mm(2) + den(1) + bc(1) + o(1)
                    #  = 5 banks alongside psK's 3)
                    for b in range(BC):
                        for st in range(KO):
                            pst = psK.tile([P, H], DT, tag="pst", name="pst")
                            for hb in range(KO):
                                nc.tensor.transpose(
                                    pst[:, hb * P:(hb + 1) * P],
                                    h4[:, hb * BS + b * S + st * P:
                                       hb * BS + b * S + (st + 1) * P],
                                    ident[:])
                            dst = h4T[:, (b * KO + st) * H:
                                      (b * KO + st + 1) * H]
                            # alternate evacuation engine to halve the
                            # psum->sbuf stage latency
                            if st % 2 == 0:
                                nc.scalar.activation(dst, pst[:], AFT.Copy)
                            else:
                                nc.vector.tensor_copy(dst, pst[:])

                    # ---- q at the last position (QSCALE lives in wkd) ----
                    ps_q = psB.tile([P, 2 * NH], F32, tag="mm",
                                    name="ps_q", bufs=2)
                    for nt in range(KO):
                        for ko in range(KO):
                            nc.tensor.matmul(
                                ps_q[:, nt * BC: (nt + 1) * BC],
                                ip_tiles["ipq"][:, ko * H + nt * P:
                                                ko * H + (nt + 1) * P],
                                lastq[:, ko * BC: (ko + 1) * BC],
                                start=(ko == 0), stop=(ko == KO - 1))
                    nc.vector.tensor_add(q_sb[:], ps_q[:], ipb_sb[:])

                    # ---- qe_j = Wk_j^T q_j  (per head, [H] each) ----
                    for no in range(KO):
                        ps_qe = psB.tile([P, 2 * NH], F32, tag="mm",
                                         name="ps_qe", bufs=2)
                        for j in range(NH):
                            nc.tensor.matmul(
                                ps_qe[:, j * BC: (j + 1) * BC],
                                ip_tiles["wkd"][:, j * H + no * P:
                                                j * H + (no + 1) * P],
                                q_sb[:, j * BC: (j + 1) * BC],
                                start=True, stop=True)
                        nc.scalar.activation(
                            qe_sb[:, no * 2 * NH: (no + 1) * 2 * NH],
                            ps_qe[:], AFT.Copy)

                    # ---- scores in [j-part, s-free]: qe stationary (cheap
                    # ldweights), h4 moving 512-wide; softmax denominator is
                    # then a per-partition scalar (j on partitions) ----
                    for b in range(BC):
                        for ch2 in range(2):
                            ps_sc = psB.tile([P, 512], F32, tag="sc",
                                             name="ps_sc", bufs=2)
                            for ko in range(KO):
                                nc.tensor.matmul(
                                    ps_sc[:NH, :],
                                    qe_sb[:, ko * 2 * NH + b:
                                          (ko + 1) * 2 * NH: BC],
                                    h4[:, ko * BS + b * S + ch2 * 512:
                                       ko * BS + b * S + (ch2 + 1) * 512],
                                    start=(ko == 0), stop=(ko == KO - 1))
                            bc2 = b * 2 + ch2
                            nc.scalar.activation(
                                e_n[:, bc2 * 512: (bc2 + 1) * 512],
                                ps_sc[:NH, :], AFT.Exp,
                                accum_out=acc2[:, bc2: bc2 + 1])
                    # denominators + normalize (j is the partition dim here)
                    nc.vector.tensor_add(dL[:], acc2[:, 0::2], acc2[:, 1::2])
                    nc.vector.reciprocal(drL[:], dL[:])
                    for b in range(BC):
                        nc.vector.tensor_scalar_mul(
                            e_n[:, b * S: (b + 1) * S],
                            e_n[:, b * S: (b + 1) * S], drL[:, b: b + 1])
                    # transpose e back to [s-part, j] tiles for the z stage
                    for b in range(BC):
                        ps_et = psB.tile([P, KO * NH], DT, tag="et",
                                         name="ps_et", bufs=1)
                        for st in range(KO):
                            nc.tensor.transpose(
                                ps_et[:, st * NH: (st + 1) * NH],
                                e_n[:NH, b * S + st * P: b * S + (st + 1) * P],
                                ident[:NH, :NH])
                        nc.scalar.activation(
                            eT_sb[:, b * KO * NH: (b + 1) * KO * NH],
                            ps_et[:], AFT.Copy)

                    # ---- z = sum_s e_s * h4_s (unnormalized) ----
                    for b in range(BC):
                        for ko in range(KO):
                            ps_zt = psB.tile([P, 2 * NH], F32, tag="mm",
                                             name="ps_z", bufs=2)
                            ps_z = ps_zt[:, :NH]
                            for st in range(KO):
                                nc.tensor.matmul(
                                    ps_z[:],
                                    h4T[:, (b * KO + st) * H + ko * P:
                                        (b * KO + st) * H + (ko + 1) * P],
                                    eT_sb[:, (b * KO + st) * NH:
                                          (b * KO + st + 1) * NH],
                                    start=(st == 0), stop=(st == KO - 1))
                            # strided write: z col = ko*16 + j*2 + b
                            nc.scalar.activation(
                                z_sb[:, ko * 2 * NH + b:
                                     (ko + 1) * 2 * NH: BC],
                                ps_z[:], AFT.Copy)

                    # ---- o_j = Wv_j^T z_j, normalized + v bias ----
                    ps_o = psB.tile([P, 2 * NH], F32, tag="o", name="ps_o")
                    for j in range(NH):
                        for ko in range(KO):
                            nc.tensor.matmul(
                                ps_o[:, j * BC: (j + 1) * BC],
                                ip_tiles["wvT"][:, ko * H + j * P:
                                                ko * H + (j + 1) * P],
                                z_sb[:, ko * 2 * NH + j * BC:
                                     ko * 2 * NH + (j + 1) * BC],
                                start=(ko == 0), stop=(ko == KO - 1))
                    nc.vector.tensor_add(O_last[:], ps_o[:], vb_sb[:])

                    # ---- out projection at last position + residual ----
                    ps_p = psB.tile([P, 2 * NH], F32, tag="mm",
                                    name="ps_p", bufs=2)
                    for no in range(KO):
                        for ko in range(KO):
                            nc.tensor.matmul(
                                ps_p[:, no * BC: (no + 1) * BC],
                                ow_sb[:, ko * H + no * P: ko * H + (no + 1) * P],
                                O_last[:, ko * BC: (ko + 1) * BC],
                                start=(ko == 0), stop=(ko == KO - 1))
                    nc.vector.tensor_add(out_last[:], ps_p[:], ob_sb[:])
                    nc.vector.tensor_add(out_last[:], out_last[:], lastq[:])
                    # ---- final fc ----
                    for ot in range(O // P):
                        ps_ft = psB.tile([P, 2 * NH], F32, tag="mm",
                                         name="ps_f", bufs=2)
                        ps_f = ps_ft[:, :BC]
                        for ko in range(KO):
                            nc.tensor.matmul(
                                ps_f[:],
                                fcw_sb[:, ko * O + ot * P: ko * O + (ot + 1) * P],
                                out_last[:, ko * BC: (ko + 1) * BC],
                                start=(ko == 0), stop=(ko == KO - 1))
                        nc.scalar.activation(
                            res_sb[:, ot * BC: (ot + 1) * BC], ps_f[:],
                            AFT.Identity, bias=fcb_sb[:, ot:ot + 1])
                    outT_v = outT.rearrange("(o p) b -> p o b", o=O // P)
                    res_v = res_sb.rearrange("p (o b) -> p o b", o=O // P)
                    nc.sync.dma_start(outT_v[:, :, :], res_v[:, :, :])

            owp_mgr.__exit__(None, None, None)
            gwp_mgr.__exit__(None, None, None)

    nc.compile()
    return nc


def _feature_major(w_t):
    """[H_in, N] (already transposed weight) -> device layout [128, KO*N]."""
    hin, n = w_t.shape
    ko = hin // P
    return np.ascontiguousarray(
        w_t.reshape(ko, P, n).transpose(1, 0, 2).reshape(P, ko * n))


def _prep_inputs(x, Wf, bf, Wi, bi, Wh, bh, in_proj_w, in_proj_b, out_w,
                 out_b, fc_w, fc_b):
    gws = []
    gbs = []
    for l in range(L):
        for W, bias in ((Wf[l], bf[l]), (Wi[l], bi[l]), (Wh[l], bh[l])):
            gws.append(_feature_major(W.T.astype(np.float32)).astype(BF16))
            gbs.append(bias.reshape(KO, P).T.astype(np.float32))
    gw = np.concatenate(gws, axis=0)                     # [12*128, KO*H]
    gb = np.concatenate(gbs, axis=1)                     # [128, 12*KO]
    # ip = [ipq | wkd | wvT]
    ipq = _feature_major(in_proj_w[:H].T.astype(np.float32))
    wkd = np.ascontiguousarray(
        in_proj_w[H:2 * H].astype(np.float32)
        .reshape(NH, P, H).transpose(1, 0, 2).reshape(P, NH * H)) * QSCALE
    wvT = _feature_major(in_proj_w[2 * H:3 * H].T.astype(np.float32))
    ip = np.concatenate([ipq, wkd, wvT], axis=1).astype(BF16)
    ipb = np.repeat(in_proj_b[:H].reshape(KO, P).T.astype(np.float32),
                    BC, axis=1)                          # col nt*BC+b
    vbv = np.repeat(in_proj_b[2 * H:].reshape(NH, P).T.astype(np.float32),
                    BC, axis=1)                          # col j*BC+b
    owp = _feature_major(out_w.T.astype(np.float32)).astype(BF16)
    obv = np.repeat(out_b.reshape(KO, P).T.astype(np.float32), BC, axis=1)
    fcwp = _feature_major(fc_w.T.astype(np.float32)).astype(BF16)
    fcbv = fc_b.reshape(O // P, P).T.astype(np.float32)
    shared = dict(gw=gw, gb=np.ascontiguousarray(gb),
                  ip=ip, ipb=np.ascontiguousarray(ipb),
                  vb=np.ascontiguousarray(vbv), ow=owp,
                  ob=np.ascontiguousarray(obv), fcw=fcwp,
                  fcb=np.ascontiguousarray(fcbv))
    in_maps = []
    for c in range(NCORES):
        shard = x[c * BC:(c + 1) * BC]                   # [BC, S, H]
        xt = shard.transpose(2, 0, 1).reshape(H, BS)     # [H, BS]
        xt = _feature_major(xt)                          # [128, KO*BS]
        # chunk-major: [p, ch(4), ko(8), 512] so chunk DMAs are contiguous
        xt = np.ascontiguousarray(
            xt.reshape(P, KO, 4, 512).transpose(0, 2, 1, 3)
            .reshape(P, KO * BS)).astype(BF16)
        in_maps.append(dict(shared, xT=xt))
    return in_maps


def kernel(x, Wf, bf, Wi, bi, Wh, bh, in_proj_w, in_proj_b, out_w, out_b,
           fc_w, fc_b):
    from concourse.bass_utils import run_bass_kernel_spmd

    x, Wf, bf, Wi, bi, Wh, bh = (np.asarray(t) for t in
                                 (x, Wf, bf, Wi, bi, Wh, bh))
    in_proj_w, in_proj_b, out_w, out_b, fc_w, fc_b = (
        np.asarray(t) for t in (in_proj_w, in_proj_b, out_w, out_b,
                                fc_w, fc_b))
    if "nc" not in _CACHE:
        _CACHE["nc"] = _build_nc()
    nc = _CACHE["nc"]
    in_maps = _prep_inputs(x, Wf, bf, Wi, bi, Wh, bh, in_proj_w, in_proj_b,
                           out_w, out_b, fc_w, fc_b)
    res = run_bass_kernel_spmd(nc, in_maps, core_ids=list(range(NCORES)))
    _CACHE["last_results"] = res
    out = np.empty((B, O), np.float32)
    for c in range(NCORES):
        outT = res.results[c]["outT"]                    # [O, BC]
        for b in range(BC):
            out[c * BC + b] = outT[:, b]
    return out


# revision 28
# speedup vs baseline: 1.0232x; 1.0211x over previous
"""Trainium2 Bass kernel for DeepMinAttLSTM (4x minLSTM + MHSA + last-step FC).

Strategy:
  - Data-parallel over batch: 16 batches -> 8 cores x 2 batches.
  - Everything on device is kept feature-major: activations live as
    X^T [H=1024 (8 partition-tiles of 128), B*S=2048 free] in bf16.
  - Per layer: 3 gate matmuls (W^T stationary, X^T moving, fp32 PSUM),
    sigmoid gates on ACT, fp/add gate math on DVE (reciprocal_approx_fast),
    and the sequential minLSTM recurrence via the DVE tensor_tensor_scan
    instruction (state fp32) along the free (time) dimension.
  - The final output only uses out[:, -1, :], so attention collapses to the
    last query position AND the K/V full-sequence GEMMs are reassociated
    away:
      scores_s = (Wk_j^T q_j) . h4_s   (k-bias drops: softmax shift-invar.)
      z_j      = sum_s e_s * h4_s      (via PE-transposed h4)
      o_j      = Wv_j^T z_j / den + bv_j
    This removes ~2*[2048,1024]x[1024,1024] matmuls per core (~110us PE).
  - All matmuls in bf16 with fp32 accumulation (predicted rel err ~7e-3).
"""

import math

import numpy as np
import ml_dtypes

BF16 = ml_dtypes.bfloat16

P = 128
H = 1024
S = 1024
B = 16
NCORES = 8
BC = B // NCORES          # batches per core
BS = BC * S               # 2048 free columns per core
KO = H // P               # 8 feature partition-tiles
NH = 8
DH = H // NH              # 128
O = 256
L = 4
QSCALE = 1.0 / math.sqrt(DH)

_CACHE = {}


def _build_nc():
    import concourse.mybir as mybir
    import concourse.tile as tile
    from concourse import bacc
    from concourse import masks

    DT = mybir.dt.bfloat16
    F32 = mybir.dt.float32
    AFT = mybir.ActivationFunctionType
    OP = mybir.AluOpType

    nc = bacc.Bacc("TRN2", target_bir_lowering=False, debug=False,
                   num_devices=NCORES)

    xT = nc.dram_tensor("xT", [P, KO * BS], DT, kind="ExternalInput").ap()
    gw = nc.dram_tensor("gw", [3 * L * P, KO * H], DT, kind="ExternalInput").ap()
    gb = nc.dram_tensor("gb", [P, 3 * L * KO], F32, kind="ExternalInput").ap()
    # ip = [ipq (KO*H) | wkd (NH*H) | wvT (KO*H)]
    ip = nc.dram_tensor("ip", [P, KO * 3 * H], DT, kind="ExternalInput").ap()
    ipb = nc.dram_tensor("ipb", [P, BC * KO], F32, kind="ExternalInput").ap()
    vb = nc.dram_tensor("vb", [P, BC * NH], F32, kind="ExternalInput").ap()
    ow = nc.dram_tensor("ow", [P, KO * H], DT, kind="ExternalInput").ap()
    ob = nc.dram_tensor("ob", [P, BC * KO], F32, kind="ExternalInput").ap()
    fcw = nc.dram_tensor("fcw", [P, KO * O], DT, kind="ExternalInput").ap()
    fcb = nc.dram_tensor("fcb", [P, O // P], F32, kind="ExternalInput").ap()
    outT = nc.dram_tensor("outT", [O, BC], F32, kind="ExternalOutput").ap()

    IPQ = 0           # ip column offsets
    WKD = KO * H
    WVT = 2 * KO * H

    with tile.TileContext(nc) as tc:
        with (
            tc.tile_pool(name="constp", bufs=1) as constp,
            tc.tile_pool(name="hbuf", bufs=2) as hp,
        ):
            gb_sb = constp.tile([P, 3 * L * KO], F32)
            nc.sync.dma_start(gb_sb[:], gb[:])
            ones_col = constp.tile([P, 1], DT)
            nc.vector.memset(ones_col[:], 1.0)
            ones_row = constp.tile([1, P], F32)
            nc.vector.memset(ones_row[:], 1.0)
            ident = constp.tile([P, P], DT)
            masks.make_identity(nc, ident[:])

            # layer-0 input is CHUNK-major ([ch, ko, 512]) so each chunk
            # DMA is one contiguous 8KB-per-partition run (DMA packet
            # efficiency); layers 1+ use the ko-major h layout.
            X = hp.tile([P, KO * BS], DT, tag="hbuf", name="xT_sb")

            # gw pool also carries the in_proj tiles (allocated during
            # layer 3 so their DMAs overlap layer-3 compute); it stays open
            # through the attention section.
            gwp_mgr = tc.tile_pool(name="gwp", bufs=5)
            gwp = gwp_mgr.__enter__()
            owp_mgr = tc.tile_pool(name="owp", bufs=1)
            owp = owp_mgr.__enter__()
            ip_tiles = {}
            ow_sb = owp.tile([P, KO * H], DT)
            fcw_sb = owp.tile([P, KO * O], DT)
            ipb_sb = constp.tile([P, BC * KO], F32)
            vb_sb = constp.tile([P, BC * NH], F32)
            ob_sb = constp.tile([P, BC * KO], F32)
            fcb_sb = constp.tile([P, O // P], F32)

            def chunked_dma(eng, dst, src, nch=4):
                cols = dst.shape[1]
                per = cols // nch
                for c in range(nch):
                    eng.dma_start(dst[:, c * per:(c + 1) * per],
                                  src[:, c * per:(c + 1) * per])

            # ---------------- minLSTM layers ----------------
            with (
                tc.tile_pool(name="fpp", bufs=2) as fpp,
                tc.tile_pool(name="addp", bufs=2) as addp,
                tc.tile_pool(name="tmpp", bufs=2) as tmpp,
                tc.tile_pool(name="psA", bufs=6, space="PSUM") as psA,
            ):
                for l in range(L):
                    gws = []
                    for g in range(3):
                        gws.append(gwp.tile([P, KO * H], DT, tag="gw",
                                            name=f"gw_{l}_{g}"))
                    if l == 0:
                        # Startup is total-HBM-bandwidth bound: issue the
                        # transfers in NEED order (x0, g0, g1, g2, x1..x3),
                        # striped round-robin over the three DMA rings, so
                        # the first psum groups can start ~3MB in instead of
                        # waiting for everything.
                        CW = KO * 512            # one chunk = 4096 cols
                        need = [(X[:, 0:CW // 2], xT[:, 0:CW // 2]),
                                (X[:, CW // 2:CW], xT[:, CW // 2:CW])]
                        for g in range(3):
                            gsrc = gw[g * P:(g + 1) * P, :]
                            for c in range(4):
                                need.append((gws[g][:, c * 2048:
                                                    (c + 1) * 2048],
                                             gsrc[:, c * 2048:
                                                  (c + 1) * 2048]))
                        for ch in range(1, 4):
                            need.append((X[:, ch * CW:(ch + 1) * CW],
                                         xT[:, ch * CW:(ch + 1) * CW]))
                        rings = [nc.sync, nc.scalar, nc.gpsimd]
                        for i, (dst, srcap) in enumerate(need):
                            rings[i % 3].dma_start(dst, srcap)
                        # attention weights/biases: plenty of DMA slack in
                        # the layer-0 window; keeps the l2->l3 boundary and
                        # the layer-3 window free for gw/in_proj traffic
                        chunked_dma(nc.sync, ow_sb[:], ow[:])
                        nc.scalar.dma_start(fcw_sb[:], fcw[:])
                        nc.scalar.dma_start(ipb_sb[:], ipb[:])
                        nc.scalar.dma_start(vb_sb[:], vb[:])
                        nc.scalar.dma_start(ob_sb[:], ob[:])
                        nc.scalar.dma_start(fcb_sb[:], fcb[:])
                    else:
                        for g in range(3):
                            lg = l * 3 + g
                            chunked_dma(nc.gpsimd, gws[g][:],
                                        gw[lg * P:(lg + 1) * P, :])
                    if l == L - 1:
                        # in_proj tiles ride the gw pool rotation: ipq/wkd
                        # reuse buffers freed at the start of layer 3; wvT
                        # reuses l3g0's buffer (freed at layer 3's end, and
                        # wvT is only needed late in the attention tail).
                        for nm, off in (("ipq", IPQ), ("wkd", WKD),
                                        ("wvT", WVT)):
                            t = gwp.tile([P, KO * H], DT, tag="gw", name=nm)
                            chunked_dma(nc.gpsimd, t[:],
                                        ip[:, off: off + KO * H])
                            ip_tiles[nm] = t
                    h_out = hp.tile([P, KO * BS], DT, tag="hbuf", name=f"h_{l}")
                    for no in range(KO):
                        fp_t = fpp.tile([P, BS], DT, tag="fp",
                                        name=f"fp_{l}_{no}")
                        add_t = addp.tile([P, BS], DT, tag="add",
                                          name=f"add_{l}_{no}")
                        for ch in range(4):
                            m0 = ch * 512
                            psF = psA.tile([P, 512], F32, tag="ps", name="psF")
                            psI = psA.tile([P, 512], F32, tag="ps", name="psI")
                            psH = psA.tile([P, 512], F32, tag="ps", name="psH")
                            for g, ps in ((0, psF), (1, psI), (2, psH)):
                                for ko in range(KO):
                                    if l == 0:
                                        xs = ch * KO * 512 + ko * 512
                                    else:
                                        xs = ko * BS + m0
                                    nc.tensor.matmul(
                                        ps[:],
                                        gws[g][:, ko * H + no * P:
                                               ko * H + (no + 1) * P],
                                        X[:, xs: xs + 512],
                                        start=(ko == 0), stop=(ko == KO - 1))
                            f_t = tmpp.tile([P, 512], DT, tag="f_t", name="f_t")
                            i_t = tmpp.tile([P, 512], DT, tag="i_t", name="i_t")
                            d_t = tmpp.tile([P, 512], F32, tag="d_t", name="d_t")
                            r_t = tmpp.tile([P, 512], F32, tag="r_t", name="r_t")
                            t1 = tmpp.tile([P, 512], F32, tag="t1", name="t1",
                                           bufs=1)
                            bF = gb_sb[:, (l * 3 + 0) * KO + no:
                                       (l * 3 + 0) * KO + no + 1]
                            bI = gb_sb[:, (l * 3 + 1) * KO + no:
                                       (l * 3 + 1) * KO + no + 1]
                            bH = gb_sb[:, (l * 3 + 2) * KO + no:
                                       (l * 3 + 2) * KO + no + 1]
                            nc.scalar.activation(f_t[:], psF[:], AFT.Sigmoid,
                                                 bias=bF)
                            nc.scalar.activation(i_t[:], psI[:], AFT.Sigmoid,
                                                 bias=bI)
                            nc.vector.tensor_add(d_t[:], f_t[:], i_t[:])
                            nc.vector.reciprocal_approx_fast(r_t[:], d_t[:])
                            nc.vector.tensor_mul(
                                fp_t[:, m0:m0 + 512], f_t[:], r_t[:])
                            # t1 = (ht_psum + bh) * r
                            nc.vector.scalar_tensor_tensor(
                                t1[:], psH[:], bH, r_t[:],
                                op0=OP.add, op1=OP.mult)
                            nc.vector.tensor_mul(
                                add_t[:, m0:m0 + 512], t1[:], i_t[:])
                            # recurrence for batch b runs as one [128,1024]
                            # scan once both of its chunks' gate math exists
                            # (scan instr overhead ~550ns, so fewer+bigger)
                            b, half = ch // 2, ch % 2
                            base = no * BS + b * S
                            if half == 1:
                                nc.vector.tensor_tensor_scan(
                                    h_out[:, base: base + S],
                                    fp_t[:, b * S: (b + 1) * S],
                                    add_t[:, b * S: (b + 1) * S],
                                    initial=0.0, op0=OP.mult, op1=OP.add)
                    X = h_out

            h4 = X

            # ---------------- attention (last query position only) ----------
            with (
                tc.tile_pool(name="smallp", bufs=1) as smallp,
            ):
                h4T = hp.tile([P, BC * KO * H], DT, tag="hbuf",
                              name="h4T")  # [s-part, (b, st, h)]
                lastq = smallp.tile([P, BC * KO], DT)
                q_sb = smallp.tile([P, BC * KO], DT)
                qe_sb = smallp.tile([P, KO * 2 * NH], DT)  # col no*16+b*8+j
                eT_sb = smallp.tile([P, BC * KO * NH], DT)  # col (b*8+kt)*8+j
                z_sb = smallp.tile([P, KO * 2 * NH], DT)   # col ko*16+j*2+b
                e_n = smallp.tile([NH, BC * S], DT)        # [j, (b,s)]
                acc2 = smallp.tile([NH, 2 * BC], F32)      # col b*2+ch
                dL = smallp.tile([NH, BC], F32)
                drL = smallp.tile([NH, BC], F32)
                O_last = smallp.tile([P, 2 * KO], DT)      # col j*BC+b
                out_last = smallp.tile([P, 2 * KO], DT)
                res_sb = smallp.tile([P, 2 * (O // P)], F32)

                # h4 columns at the last timestep (per ko-tile, per batch)
                for ko in range(KO):
                    for b in range(BC):
                        nc.vector.tensor_copy(
                            lastq[:, ko * BC + b: ko * BC + b + 1],
                            h4[:, ko * BS + b * S + S - 1:
                               ko * BS + b * S + S])

                with (
                    tc.tile_pool(name="psK", bufs=2, space="PSUM") as psK,
                    tc.tile_pool(name="psB", bufs=1, space="PSUM") as psB,
                ):
                    # ---- h4T: transpose h4 into [s-part, (b, st, h)] ----
                    # (psum small-tile budget: mm(2) + den(1) + bc(1) + o(1)
                    #  = 5 banks alongside psK's 3)
                    for b in range(BC):
                        for st in range(KO):
                            pst = psK.tile([P, H], DT, tag="pst", name="pst")
                            for hb in range(KO):
                                nc.tensor.transpose(
                                    pst[:, hb * P:(hb + 1) * P],
                                    h4[:, hb * BS + b * S + st * P:
                                       hb * BS + b * S + (st + 1) * P],
                                    ident[:])
                            dst = h4T[:, (b * KO + st) * H:
                                      (b * KO + st + 1) * H]
                            # alternate evacuation engine to halve the
                            # psum->sbuf stage latency
                            if st % 2 == 0:
                                nc.scalar.activation(dst, pst[:], AFT.Copy)
                            else:
                                nc.vector.tensor_copy(dst, pst[:])

                    # ---- q at the last position (QSCALE lives in wkd) ----
                    ps_q = psB.tile([P, 2 * NH], F32, tag="mm",
                                    name="ps_q", bufs=2)
                    for nt in range(KO):
                        for ko in range(KO):
                            nc.tensor.matmul(
                                ps_q[:, nt * BC: (nt + 1) * BC],
                                ip_tiles["ipq"][:, ko * H + nt * P:
                                                ko * H + (nt + 1) * P],
                                lastq[:, ko * BC: (ko + 1) * BC],
                                start=(ko == 0), stop=(ko == KO - 1))
                    nc.vector.tensor_add(q_sb[:], ps_q[:], ipb_sb[:])

                    # ---- qe_j = Wk_j^T q_j  (per head, [H] each) ----
                    for no in range(KO):
                        ps_qe = psB.tile([P, 2 * NH], F32, tag="mm",
                                         name="ps_qe", bufs=2)
                        for j in range(NH):
                            nc.tensor.matmul(
                                ps_qe[:, j * BC: (j + 1) * BC],
                                ip_tiles["wkd"][:, j * H + no * P:
                                                j * H + (no + 1) * P],
                                q_sb[:, j * BC: (j + 1) * BC],
                                start=True, stop=True)
                        nc.scalar.activation(
                            qe_sb[:, no * 2 * NH: (no + 1) * 2 * NH],
                            ps_qe[:], AFT.Copy)

                    # ---- scores in [j-part, s-free]: qe stationary (cheap
                    # ldweights), h4 moving 512-wide; softmax denominator is
                    # then a per-partition scalar (j on partitions) ----
                    for b in range(BC):
                        for ch2 in range(2):
                            ps_sc = psB.tile([P, 512], F32, tag="sc",
                                             name="ps_sc", bufs=2)
                            for ko in range(KO):
                                nc.tensor.matmul(
                                    ps_sc[:NH, :],
                                    qe_sb[:, ko * 2 * NH + b:
                                          (ko + 1) * 2 * NH: BC],
                                    h4[:, ko * BS + b * S + ch2 * 512:
                                       ko * BS + b * S + (ch2 + 1) * 512],
                                    start=(ko == 0), stop=(ko == KO - 1))
                            bc2 = b * 2 + ch2
                            nc.scalar.activation(
                                e_n[:, bc2 * 512: (bc2 + 1) * 512],
                                ps_sc[:NH, :], AFT.Exp,
                                accum_out=acc2[:, bc2: bc2 + 1])
                    # denominators + normalize (j is the partition dim here)
                    nc.vector.tensor_add(dL[:], acc2[:, 0::2], acc2[:, 1::2])
                    nc.vector.reciprocal(drL[:], dL[:])
                    for b in range(BC):
                        nc.vector.tensor_scalar_mul(
                            e_n[:, b * S: (b + 1) * S],
                            e_n[:, b * S: (b + 1) * S], drL[:, b: b + 1])
                    # transpose e back to [s-part, j] tiles for the z stage
                    for b in range(BC):
                        ps_et = psB.tile([P, KO * NH], DT, tag="et",
                                         name="ps_et", bufs=1)
                        for st in range(KO):
                            nc.tensor.transpose(
                                ps_et[:, st * NH: (st + 1) * NH],
                                e_n[:NH, b * S + st * P: b * S + (st + 1) * P],
                                ident[:NH, :NH])
                        nc.scalar.activation(
                            eT_sb[:, b * KO * NH: (b + 1) * KO * NH],
                            ps_et[:], AFT.Copy)

                    # ---- z = sum_s e_s * h4_s (unnormalized) ----
                    for b in range(BC):
                        for ko in range(KO):
                            ps_zt = psB.tile([P, 2 * NH], F32, tag="mm",
                                             name="ps_z", bufs=2)
                            ps_z = ps_zt[:, :NH]
                            for st in range(KO):
                                nc.tensor.matmul(
                                    ps_z[:],
                                    h4T[:, (b * KO + st) * H + ko * P:
                                        (b * KO + st) * H + (ko + 1) * P],
                                    eT_sb[:, (b * KO + st) * NH:
                                          (b * KO + st + 1) * NH],
                                    start=(st == 0), stop=(st == KO - 1))
                            # strided write: z col = ko*16 + j*2 + b
                            nc.scalar.activation(
                                z_sb[:, ko * 2 * NH + b:
                                     (ko + 1) * 2 * NH: BC],
                                ps_z[:], AFT.Copy)

                    # ---- o_j = Wv_j^T z_j, normalized + v bias ----
                    ps_o = psB.tile([P, 2 * NH], F32, tag="o", name="ps_o")
                    for j in range(NH):
                        for ko in range(KO):
                            nc.tensor.matmul(
                                ps_o[:, j * BC: (j + 1) * BC],
                                ip_tiles["wvT"][:, ko * H + j * P:
                                                ko * H + (j + 1) * P],
                                z_sb[:, ko * 2 * NH + j * BC:
                                     ko * 2 * NH + (j + 1) * BC],
                                start=(ko == 0), stop=(ko == KO - 1))
                    nc.vector.tensor_add(O_last[:], ps_o[:], vb_sb[:])

                    # ---- out projection at last position + residual ----
                    ps_p = psB.tile([P, 2 * NH], F32, tag="mm",
                                    name="ps_p", bufs=2)
                    for no in range(KO):
                        for ko in range(KO):
                            nc.tensor.matmul(
                                ps_p[:, no * BC: (no + 1) * BC],
                                ow_sb[:, ko * H + no * P: ko * H + (no + 1) * P],
                                O_last[:, ko * BC: (ko + 1) * BC],
                                start=(ko == 0), stop=(ko == KO - 1))
                    nc.vector.tensor_add(out_last[:], ps_p[:], ob_sb[:])
                    nc.vector.tensor_add(out_last[:], out_last[:], lastq[:])
                    # ---- final fc ----
                    for ot in range(O // P):
                        ps_ft = psB.tile([P, 2 * NH], F32, tag="mm",
                                         name="ps_f", bufs=2)
                        ps_f = ps_ft[:, :BC]
                        for ko in range(KO):
                            nc.tensor.matmul(
                                ps_f[:],
                                fcw_sb[:, ko * O + ot * P: ko * O + (ot + 1) * P],
                                out_last[:, ko * BC: (ko + 1) * BC],
                                start=(ko == 0), stop=(ko == KO - 1))
                        nc.scalar.activation(
                            res_sb[:, ot * BC: (ot + 1) * BC], ps_f[:],
                            AFT.Identity, bias=fcb_sb[:, ot:ot + 1])
                    outT_v = outT.rearrange("(o p) b -> p o b", o=O // P)
                    res_v = res_sb.rearrange("p (o b) -> p o b", o=O // P)
                    nc.sync.dma_start(outT_v[:, :, :], res_v[:, :, :])

            owp_mgr.__exit__(None, None, None)
            gwp_mgr.__exit__(None, None, None)

    nc.compile()
    return nc


def _feature_major(w_t):
    """[H_in, N] (already transposed weight) -> device layout [128, KO*N]."""
    hin, n = w_t.shape
    ko = hin // P
    return np.ascontiguousarray(
        w_t.reshape(ko, P, n).transpose(1, 0, 2).reshape(P, ko * n))


def _prep_inputs(x, Wf, bf, Wi, bi, Wh, bh, in_proj_w, in_proj_b, out_w,
                 out_b, fc_w, fc_b):
    gws = []
    gbs = []
    for l in range(L):
        for W, bias in ((Wf[l], bf[l]), (Wi[l], bi[l]), (Wh[l], bh[l])):
            gws.append(_feature_major(W.T.astype(np.float32)).astype(BF16))
            gbs.append(bias.reshape(KO, P).T.astype(np.float32))
    gw = np.concatenate(gws, axis=0)                     # [12*128, KO*H]
    gb = np.concatenate(gbs, axis=1)                     # [128, 12*KO]
    # ip = [ipq | wkd | wvT]
    ipq = _feature_major(in_proj_w[:H].T.astype(np.float32))
    wkd = np.ascontiguousarray(
        in_proj_w[H:2 * H].astype(np.float32)
        .reshape(NH, P, H).transpose(1, 0, 2).reshape(P, NH * H)) * QSCALE
    wvT = _feature_major(in_proj_w[2 * H:3 * H].T.astype(np.float32))
    ip = np.concatenate([ipq, wkd, wvT], axis=1).astype(BF16)
    ipb = np.repeat(in_proj_b[:H].reshape(KO, P).T.astype(np.float32),
                    BC, axis=1)                          # col nt*BC+b
    vbv = np.repeat(in_proj_b[2 * H:].reshape(NH, P).T.astype(np.float32),
                    BC, axis=1)                          # col j*BC+b
    owp = _feature_major(out_w.T.astype(np.float32)).astype(BF16)
    obv = np.repeat(out_b.reshape(KO, P).T.astype(np.float32), BC, axis=1)
    fcwp = _feature_major(fc_w.T.astype(np.float32)).astype(BF16)
    fcbv = fc_b.reshape(O // P, P).T.astype(np.float32)
    shared = dict(gw=gw, gb=np.ascontiguousarray(gb),
                  ip=ip, ipb=np.ascontiguousarray(ipb),
                  vb=np.ascontiguousarray(vbv), ow=owp,
                  ob=np.ascontiguousarray(obv), fcw=fcwp,
                  fcb=np.ascontiguousarray(fcbv))
    in_maps = []
    for c in range(NCORES):
        shard = x[c * BC:(c + 1) * BC]                   # [BC, S, H]
        xt = shard.transpose(2, 0, 1).reshape(H, BS)     # [H, BS]
        xt = _feature_major(xt)                          # [128, KO*BS]
        # chunk-major: [p, ch(4), ko(8), 512] so chunk DMAs are contiguous
        xt = np.ascontiguousarray(
            xt.reshape(P, KO, 4, 512).transpose(0, 2, 1, 3)
            .reshape(P, KO * BS)).astype(BF16)
        in_maps.append(dict(shared, xT=xt))
    return in_maps


def kernel(x, Wf, bf, Wi, bi, Wh, bh, in_proj_w, in_proj_b, out_w, out_b,
           fc_w, fc_b):
    from concourse.bass_utils import run_bass_kernel_spmd

    x, Wf, bf, Wi, bi, Wh, bh = (np.asarray(t) for t in
                                 (x, Wf, bf, Wi, bi, Wh, bh))
    in_proj_w, in_proj_b, out_w, out_b, fc_w, fc_b = (
        np.asarray(t) for t in (in_proj_w, in_proj_b, out_w, out_b,
                                fc_w, fc_b))
    if "nc" not in _CACHE:
        _CACHE["nc"] = _build_nc()
    nc = _CACHE["nc"]
    in_maps = _prep_inputs(x, Wf, bf, Wi, bi, Wh, bh, in_proj_w, in_proj_b,
                           out_w, out_b, fc_w, fc_b)
    res = run_bass_kernel_spmd(nc, in_maps, core_ids=list(range(NCORES)))
    _CACHE["last_results"] = res
    out = np.empty((B, O), np.float32)
    for c in range(NCORES):
        outT = res.results[c]["outT"]                    # [O, BC]
        for b in range(BC):
            out[c * BC + b] = outT[:, b]
    return out
